# revision 1
# baseline (speedup 1.0000x reference)
"""Trainium2 Bass kernel for the quantum-circuit KG-embedding scoring model.

Math: score(s,p,o) = Re(<B_o h | W_p | B_s h>) where B_e / W_p are the
24-gate circuit blocks for entity/relation params and h = |+>^6.

Device algorithm (8 cores, SPMD):
  Phase A: each core computes T[e] = B_e h for its 1280-entity shard
           (product-state doubling + 18 CRots, batched 128 entities per
           partition-tile, all-tile ops via free-dim broadcast coeffs),
           writes rows to DRAM, AllGather -> full T table [10240, 128]
           (real repr: 64 re | 64 im).
  Phase W: each core computes W_real^T for its <=32 local relations
           (two relations packed per 128-partition tile; identity basis
           columns evolved by the same gate machinery), stores to DRAM.
  Phase C: batch sorted by relation on host, padded to 128-elem tiles of
           a single relation each. Per tile: indirect-gather T rows for
           s and o, gather W^T, PE-transpose Ts/To, Y = W @ Ts (PE),
           score = ones^T (To .* Y) via PE, store scores.

Host does only: trig for the 200-relation coeff tables (tiny), index
sort/padding, and output unpermute.
"""

import sys
import numpy as np

for _p in ("/opt/trn_rl_repo",):
    if _p not in sys.path:
        sys.path.insert(0, _p)

import concourse.bass as bass
import concourse.bacc as bacc
import concourse.mybir as mybir
from concourse import tile
from concourse.bass_utils import run_bass_kernel_spmd

F32 = mybir.dt.float32
I32 = mybir.dt.int32
ALU = mybir.AluOpType
ACTFN = mybir.ActivationFunctionType

P = 128
Q = 6
NA = 64                      # 2^Q amplitudes
NCORES = 8
E, R, B = 10000, 200, 65536
ETILES = 10                  # entity tiles per core
EPC = ETILES * P             # 1280 entities per core
EPAD = EPC * NCORES          # 10240 padded entity rows
WTILES = 16                  # W-phase tiles per core (2 relations each)
RSLOT = 2 * WTILES           # 32 relation slots per core
NT = 96                      # phase-C element tiles per core
NGATE = 24
NCO = 7                      # coeff slots per gate: v0 v1 v2 v3 -v1 -v2 -v3
NGC = NGATE * NCO
R2 = float(2.0 ** -0.5)
PI = float(np.pi)

# CRot gate list: (control, target) wire pairs, in circuit order
CROTS = [(q, (q + off) % Q) for off in (1, 2, 3) for q in range(Q)]


# --------------------------------------------------------------------------
# device program
# --------------------------------------------------------------------------

def _ap_bc(coef_ap, like_ap):
    """Broadcast a [128, ntiles, 1] coeff AP to the shape of like_ap."""
    return coef_ap.to_broadcast(like_ap.shape)


def _crot_groups(st, cpos, tpos):
    """Sub-slice groups for a CRot on state [128, nt, 128].

    Each group: dict with a0r/a0i/a1r/a1i APs, every AP having <= 3
    non-trivial free dims (DVE TENSOR3D limit). Size-1 dims are removed by
    integer indexing; if 4 non-trivial dims remain, split over a size-2 dim.
    """
    hi, lo = max(cpos, tpos), min(cpos, tpos)
    A = 1 << (5 - hi)
    Bm = 1 << (hi - lo - 1)
    C = 1 << lo
    v = st.rearrange(
        "p n (r a x b y c) -> p n r a x b y c", r=2, a=A, x=2, b=Bm, y=2, c=C
    )
    cbit_is_x = cpos == hi
    nontriv = (A > 1) + (Bm > 1) + (C > 1)
    # choose split dim (by name) when 4 total free dims (incl. nt)
    split_dim = None
    if nontriv >= 3:
        for nm, sz in (("a", A), ("b", Bm), ("c", C)):
            if sz == 2:
                split_dim = nm
                break
        assert split_dim is not None, (A, Bm, C)

    def sel(r, cval, tval, sub):
        xbit, ybit = (cval, tval) if cbit_is_x else (tval, cval)
        idx = [slice(None), slice(None), r]
        # dims: a, x, b, y, c
        def dimsel(nm, sz):
            if nm == split_dim:
                return sub
            return slice(None) if sz > 1 else 0
        idx.append(dimsel("a", A))
        idx.append(xbit)
        idx.append(dimsel("b", Bm))
        idx.append(ybit)
        idx.append(dimsel("c", C))
        return v[tuple(idx)]

    subs = (0, 1) if split_dim is not None else (None,)
    groups = []
    for sub in subs:
        groups.append(
            {
                "a0r": sel(0, 1, 0, sub),
                "a0i": sel(1, 1, 0, sub),
                "a1r": sel(0, 1, 1, sub),
                "a1i": sel(1, 1, 1, sub),
            }
        )
    return groups


def _emit_crot(nc, pool, state, coef, nt, g, c, t, tmp_tag):
    """Apply CRot gate g (control wire c, target wire t) in place on state.

    coef: [128, nt, NGC]; per-gate slots at g*NCO:
      0: v0, 1: v1, 2: v2, 3: v3, 4: -v1, 5: -v2, 6: -v3
      n0r = v0*a0r + v1*a0i + (-v2)*a1r + v3*a1i
      n0i = (-v1)*a0r + v0*a0i + (-v3)*a1r + (-v2)*a1i
      n1r = v2*a0r + v3*a0i + v0*a1r + (-v1)*a1i
      n1i = (-v3)*a0r + v2*a0i + v1*a1r + v0*a1i
    """
    cpos, tpos = 5 - c, 5 - t
    groups = _crot_groups(state, cpos, tpos)

    def co(slot, like):
        apc = coef[:, :, g * NCO + slot : g * NCO + slot + 1]
        return apc.to_broadcast(like.shape)

    comp_terms = {
        "a0r": [(0, "a0r"), (1, "a0i"), (5, "a1r"), (3, "a1i")],
        "a0i": [(4, "a0r"), (0, "a0i"), (6, "a1r"), (5, "a1i")],
        "a1r": [(2, "a0r"), (3, "a0i"), (0, "a1r"), (4, "a1i")],
        "a1i": [(6, "a0r"), (2, "a0i"), (1, "a1r"), (0, "a1i")],
    }
    for gi, grp in enumerate(groups):
        shape = grp["a0r"].shape
        fsz = 1
        for d in shape[1:]:
            fsz *= d
        temps = {}
        for out_name, terms in comp_terms.items():
            tt = pool.tile([P, fsz], F32, tag=f"{tmp_tag}{out_name}")
            ta = tt[:].rearrange(
                "p (x y) -> p x y", x=shape[1], y=fsz // shape[1]
            ) if len(shape) == 3 else (
                tt[:].rearrange(
                    "p (x y z) -> p x y z",
                    x=shape[1], y=shape[2], z=shape[3],
                ) if len(shape) == 4 else tt[:]
            )
            s0, i0 = terms[0]
            nc.vector.tensor_tensor(out=ta, in0=grp[i0], in1=co(s0, grp[i0]), op=ALU.mult)
            tb_t = pool.tile([P, fsz], F32, tag=f"{tmp_tag}b")
            tb = tb_t[:].rearrange(
                "p (x y) -> p x y", x=shape[1], y=fsz // shape[1]
            ) if len(shape) == 3 else (
                tb_t[:].rearrange(
                    "p (x y z) -> p x y z",
                    x=shape[1], y=shape[2], z=shape[3],
                ) if len(shape) == 4 else tb_t[:]
            )
            for sl, inp in terms[1:]:
                nc.vector.tensor_tensor(out=tb, in0=grp[inp], in1=co(sl, grp[inp]), op=ALU.mult)
                nc.vector.tensor_tensor(out=ta, in0=ta, in1=tb, op=ALU.add)
            temps[out_name] = ta
        for out_name in comp_terms:
            nc.vector.tensor_copy(out=grp[out_name], in_=temps[out_name])


def _emit_doubling(nc, pool, sbufs, fac, nt, tag):
    """Materialize product state from factors.

    sbufs: (sA, sB) ping-pong [128, nt, 128].
    fac: [128, nt_or_1, 6, 6] with per-step slots
         [u0r, u0i, -u0i, u1r, u1i, -u1i]; step k expands wire q=5-k.
    Initial: amp = 1.0 at col 0 handled by seeding from fac step 0 directly:
       state after step0: re[0]=u0r, re[1]=u1r, im[0]=u0i, im[1]=u1i.
    Returns final state tile (one of sbufs).
    """
    sA, sB = sbufs
    cur = sA
    # seed: copy factor step 0 entries into state cols
    for (dst_col, src_slot) in ((0, 0), (1, 3)):      # re: u0r, u1r
        nc.vector.tensor_copy(
            out=cur[:, :, dst_col : dst_col + 1], in_=fac[:, :, 0, src_slot : src_slot + 1]
        )
    for (dst_col, src_slot) in ((64, 1), (65, 4)):    # im: u0i, u1i
        nc.vector.tensor_copy(
            out=cur[:, :, dst_col : dst_col + 1], in_=fac[:, :, 0, src_slot : src_slot + 1]
        )
    for k in range(1, 6):
        w = 1 << k
        nxt = sB if cur is sA else sA
        cr, ci = cur[:, :, 0:w], cur[:, :, 64 : 64 + w]
        for m in (0, 1):
            # factor (f_r, f_i) = fac[..., k, 3m], fac[..., k, 3m+1]; -f_i at 3m+2
            fr = fac[:, :, k, 3 * m : 3 * m + 1]
            fi = fac[:, :, k, 3 * m + 1 : 3 * m + 2]
            nfi = fac[:, :, k, 3 * m + 2 : 3 * m + 3]
            dr = nxt[:, :, m * w : m * w + w]
            di = nxt[:, :, 64 + m * w : 64 + m * w + w]
            tmp = pool.tile([P, nt * w], F32, tag=tag)
            tm = tmp[:].rearrange("p (n w) -> p n w", n=nt, w=w)
            # dr = cr*fr + ci*(-fi)
            nc.vector.tensor_tensor(out=tm, in0=cr, in1=fr.to_broadcast(cr.shape), op=ALU.mult)
            tmp2 = pool.tile([P, nt * w], F32, tag=tag + "b")
            tm2 = tmp2[:].rearrange("p (n w) -> p n w", n=nt, w=w)
            nc.vector.tensor_tensor(out=tm2, in0=ci, in1=nfi.to_broadcast(ci.shape), op=ALU.mult)
            nc.vector.tensor_tensor(out=dr, in0=tm, in1=tm2, op=ALU.add)
            # di = cr*fi + ci*fr
            nc.vector.tensor_tensor(out=tm, in0=cr, in1=fi.to_broadcast(cr.shape), op=ALU.mult)
            nc.vector.tensor_tensor(out=tm2, in0=ci, in1=fr.to_broadcast(ci.shape), op=ALU.mult)
            nc.vector.tensor_tensor(out=di, in0=tm, in1=tm2, op=ALU.add)
        cur = nxt
    return cur


def build_program(no_collective=False, stop_after=None):
    """no_collective=True builds a TimelineSim-compatible variant (the
    AllGather is replaced by a local DRAM copy; timing-equivalent except the
    collective itself, correctness-invalid for multi-core)."""
    nc = bacc.Bacc("TRN2", target_bir_lowering=False, debug=False)

    ent = nc.dram_tensor("ent_par", [ETILES, P, 72], F32, kind="ExternalInput")
    wcoef_d = nc.dram_tensor("wcoef", [P, WTILES, NGC], F32, kind="ExternalInput")
    wfac_d = nc.dram_tensor("wfac", [P, WTILES, 6, 6], F32, kind="ExternalInput")
    sidx_d = nc.dram_tensor("sidx", [NT, P], I32, kind="ExternalInput")
    oidx_d = nc.dram_tensor("oidx", [NT, P], I32, kind="ExternalInput")
    widx_d = nc.dram_tensor("widx", [NT, P], I32, kind="ExternalInput")
    ident_d = nc.dram_tensor("ident", [P, P], F32, kind="ExternalInput")
    scores_d = nc.dram_tensor("scores", [P, NT], F32, kind="ExternalOutput")

    with tile.TileContext(nc) as tc:
        with (
            tc.tile_pool(name="const", bufs=1) as cp,
            tc.tile_pool(name="gtmp", bufs=2) as gp,
            tc.tile_pool(name="state", bufs=1) as sp,
            tc.tile_pool(name="cbuf", bufs=3) as cb,
            tc.tile_pool(name="cps", bufs=2, space="PSUM") as psT,
            tc.tile_pool(name="cpy", bufs=2, space="PSUM") as psY,
            tc.tile_pool(name="cpsc", bufs=2, space="PSUM") as psS,
            tc.tile_pool(name="dram", bufs=1, space="DRAM") as dp,
        ):
            # ---------------- DRAM scratch ----------------
            T_loc = dp.tile([EPC, P], F32)
            T_full = dp.tile([EPAD, P], F32, addr_space="Shared")
            W_loc = dp.tile([RSLOT * P, P], F32)

            # ---------------- load inputs to SBUF ----------------
            ang = cp.tile([P, ETILES, 72], F32)
            # dram (t, p, k) -> sbuf [p, t, k]
            nc.sync.dma_start(
                out=ang[:], in_=ent[:].rearrange("t p k -> p t k")
            )
            wcoef = cp.tile([P, WTILES, NGC], F32)
            nc.sync.dma_start(out=wcoef[:], in_=wcoef_d[:])
            wfac = cp.tile([P, WTILES, 6, 6], F32)
            nc.sync.dma_start(out=wfac[:], in_=wfac_d[:])
            sidx = cp.tile([P, NT], I32)
            nc.sync.dma_start(out=sidx[:], in_=sidx_d[:].rearrange("t p -> p t"))
            oidx = cp.tile([P, NT], I32)
            nc.sync.dma_start(out=oidx[:], in_=oidx_d[:].rearrange("t p -> p t"))
            widx = cp.tile([P, NT], I32)
            nc.sync.dma_start(out=widx[:], in_=widx_d[:].rearrange("t p -> p t"))

            ones = cp.tile([P, 1], F32)
            nc.vector.memset(ones[:], 1.0)

            # const APs for activation scale/bias floats
            cdb = cp.tile([P, 3], F32)
            nc.vector.memset(cdb[:, 0:1], 0.0)
            nc.vector.memset(cdb[:, 1:2], 0.5)
            nc.vector.memset(cdb[:, 2:3], PI / 2)
            nc.const_aps.aps[(F32, 0.0)] = cdb[:, 0:1]
            nc.const_aps.aps[(F32, 0.5)] = cdb[:, 1:2]
            nc.const_aps.aps[(F32, PI / 2)] = cdb[:, 2:3]

            # ---------------- phase A: entity coeffs ----------------
            acoef = cp.tile([P, ETILES, NGC], F32)
            afac = cp.tile([P, ETILES, 6, 6], F32)

            phi = ang[:].rearrange("p t (g a) -> p t g a", g=24, a=3)[:, :, :, 0]
            tha = ang[:].rearrange("p t (g a) -> p t g a", g=24, a=3)[:, :, :, 1]
            omg = ang[:].rearrange("p t (g a) -> p t g a", g=24, a=3)[:, :, :, 2]

            s1 = cp.tile([P, ETILES, 24], F32)   # phi + omega
            s2 = cp.tile([P, ETILES, 24], F32)   # phi - omega
            nc.vector.tensor_tensor(out=s1[:], in0=phi, in1=omg, op=ALU.add)
            nc.vector.tensor_tensor(out=s2[:], in0=phi, in1=omg, op=ALU.subtract)

            trig = cp.tile([P, ETILES, 6, 24], F32)  # ch, sh, ca, sa, cb, sb
            tv = trig[:]
            # pre-scale on DVE (keeps ACT deps single-engine), then plain Sin
            half = cp.tile([P, ETILES, 6, 24], F32)
            hv = half[:]
            for i, srcv in ((0, tha), (2, s1[:]), (4, s2[:])):
                nc.vector.tensor_scalar(
                    out=hv[:, :, i], in0=srcv, scalar1=0.5, scalar2=PI / 2,
                    op0=ALU.mult, op1=ALU.add,
                )
                nc.vector.tensor_scalar_mul(hv[:, :, i + 1], srcv, 0.5)
            for i in range(6):
                nc.scalar.activation(out=tv[:, :, i], in_=hv[:, :, i], func=ACTFN.Sin)

            av = acoef[:].rearrange("p t (g s) -> p t g s", g=24, s=NCO)
            # v0 = ch*ca, v1 = ch*sa, v2 = sh*cb, v3 = sh*sb
            nc.vector.tensor_tensor(out=av[:, :, :, 0], in0=tv[:, :, 0], in1=tv[:, :, 2], op=ALU.mult)
            nc.vector.tensor_tensor(out=av[:, :, :, 1], in0=tv[:, :, 0], in1=tv[:, :, 3], op=ALU.mult)
            nc.vector.tensor_tensor(out=av[:, :, :, 2], in0=tv[:, :, 1], in1=tv[:, :, 4], op=ALU.mult)
            nc.vector.tensor_tensor(out=av[:, :, :, 3], in0=tv[:, :, 1], in1=tv[:, :, 5], op=ALU.mult)
            for dst, src in ((4, 1), (5, 2), (6, 3)):
                nc.vector.tensor_scalar_mul(av[:, :, :, dst], av[:, :, :, src], -1.0)

            # layer-0 |+> factors: f_m = (u[m,0]+u[m,1])/sqrt(2), gates g=0..5
            # f0 = (m00+m01)/r2 = ((v0-v2) + i(-v1-v3))*r2
            # f1 = (m10+m11)/r2 = ((v2+v0) + i(v1-v3))*r2
            g6 = av[:, :, 0:6]                     # [p, t, 6, NCO]
            fv = afac[:]
            tmp6 = cp.tile([P, ETILES, 6], F32)
            # NOTE afac step k expands wire q=5-k -> use gate q=5-k of layer 0
            for k in range(6):
                qg = 5 - k
                v0 = g6[:, :, qg, 0:1]
                v1 = g6[:, :, qg, 1:2]
                v2 = g6[:, :, qg, 2:3]
                v3 = g6[:, :, qg, 3:4]
                tt = tmp6[:, :, k : k + 1]
                # f0r
                nc.vector.tensor_tensor(out=tt, in0=v0, in1=v2, op=ALU.subtract)
                nc.vector.tensor_scalar_mul(fv[:, :, k, 0:1], tt, R2)
                # f0i = -(v1+v3)*r2 ; -f0i
                nc.vector.tensor_tensor(out=tt, in0=v1, in1=v3, op=ALU.add)
                nc.vector.tensor_scalar_mul(fv[:, :, k, 1:2], tt, -R2)
                nc.vector.tensor_scalar_mul(fv[:, :, k, 2:3], tt, R2)
                # f1r
                nc.vector.tensor_tensor(out=tt, in0=v0, in1=v2, op=ALU.add)
                nc.vector.tensor_scalar_mul(fv[:, :, k, 3:4], tt, R2)
                # f1i = (v1-v3)*r2 ; -f1i
                nc.vector.tensor_tensor(out=tt, in0=v1, in1=v3, op=ALU.subtract)
                nc.vector.tensor_scalar_mul(fv[:, :, k, 4:5], tt, R2)
                nc.vector.tensor_scalar_mul(fv[:, :, k, 5:6], tt, -R2)

            # ---------------- phase A: state evolution ----------------
            sA = sp.tile([P, ETILES, P], F32)
            sB = sp.tile([P, ETILES, P], F32)
            cur = _emit_doubling(nc, gp, (sA, sB), afac[:], ETILES, "adbl")
            for g, (c, t) in enumerate(CROTS):
                _emit_crot(nc, gp, cur[:], acoef[:], ETILES, 6 + g, c, t, "acr")

            # store T rows: entity (t, p) -> row 128t + p
            nc.sync.dma_start(
                out=T_loc[:].rearrange("(t p) k -> p t k", p=P), in_=cur[:]
            )
            # AllGather T
            if no_collective:
                nc.sync.dma_start(out=T_full[0:EPC, :], in_=T_loc[:])
            else:
                nc.gpsimd.collective_compute(
                    "AllGather",
                    ALU.bypass,
                    ins=[T_loc[:]],
                    outs=[T_full[:]],
                    replica_groups=[list(range(NCORES))],
                )

            # ---------------- phase W ----------------
            if stop_after != "A":
              sW = sp.tile([P, WTILES, P], F32)
              sW2 = sp.tile([P, WTILES, P], F32)
              curw = _emit_doubling(nc, gp, (sW, sW2), wfac[:], WTILES, "wdbl")
              for g, (c, t) in enumerate(CROTS):
                  _emit_crot(nc, gp, curw[:], wcoef[:], WTILES, 6 + g, c, t, "wcr")

            # expand packed [relpair] tiles into W^T slots and store to DRAM.
            # curw tile wt: partitions 0:64 = rel slot 2wt rows j (basis j),
            # partitions 64:128 = rel slot 2wt+1. Each row: [yr(64) | yi(64)].
            # W^T slot layout [128 rows j, 128 cols i]:
            #   rows 0:64   = computed rows (top half of W^T)
            #   rows 64:128: [ -yi | yr ] of same relation (cols swapped, re negated)
              for wt in range(WTILES):
                for half in range(2):
                    slot = 2 * wt + half
                    ex = gp.tile([P, P], F32, tag="wex")
                    srcw = curw[:, wt, :]
                    h = srcw[64 * half : 64 * half + 64, :]   # [64, 128]
                    nc.vector.tensor_copy(out=ex[0:64, :], in_=h)
                    # bottom half: [-yi | yr]
                    nc.vector.tensor_scalar_mul(ex[64:128, 0:64], h[:, 64:128], -1.0)
                    nc.vector.tensor_copy(out=ex[64:128, 64:128], in_=h[:, 0:64])
                    nc.sync.dma_start(
                        out=W_loc[:].rearrange("(s p) k -> s p k", p=P)[slot], in_=ex[:]
                    )

            # ---------------- phase C ----------------
            if stop_after is None:
              scores = cp.tile([P, NT], F32)
              ident = cp.tile([P, P], F32)
              nc.sync.dma_start(out=ident[:], in_=ident_d[:])

              # per-tile compute
              for t in range(NT):
                 gTs = cb.tile([P, P], F32, tag="gts")
                 nc.gpsimd.indirect_dma_start(
                     out=gTs[:],
                     out_offset=None,
                     in_=T_full[:],
                     in_offset=bass.IndirectOffsetOnAxis(ap=sidx[:, t : t + 1], axis=0),
                 )
                 gTo = cb.tile([P, P], F32, tag="gto")
                 nc.gpsimd.indirect_dma_start(
                     out=gTo[:],
                     out_offset=None,
                     in_=T_full[:],
                     in_offset=bass.IndirectOffsetOnAxis(ap=oidx[:, t : t + 1], axis=0),
                 )
                 gW = cb.tile([P, P], F32, tag="gw")
                 nc.gpsimd.indirect_dma_start(
                     out=gW[:],
                     out_offset=None,
                     in_=W_loc[:],
                     in_offset=bass.IndirectOffsetOnAxis(ap=widx[:, t : t + 1], axis=0),
                 )
                 pTs = psT.tile([P, P], F32, tag="pts")
                 nc.tensor.transpose(out=pTs[:], in_=gTs[:], identity=ident[:])
                 sTsT = cb.tile([P, P], F32, tag="tst")
                 nc.vector.tensor_copy(out=sTsT[:], in_=pTs[:])
                 pTo = psT.tile([P, P], F32, tag="pto")
                 nc.tensor.transpose(out=pTo[:], in_=gTo[:], identity=ident[:])
                 sToT = cb.tile([P, P], F32, tag="tot")
                 nc.vector.tensor_copy(out=sToT[:], in_=pTo[:])

                 pY = psY.tile([P, P], F32, tag="py")
                 nc.tensor.matmul(out=pY[:], lhsT=gW[:], rhs=sTsT[:], start=True, stop=True)
                 sY = cb.tile([P, P], F32, tag="sy")
                 nc.vector.tensor_copy(out=sY[:], in_=pY[:])
                 prod = cb.tile([P, P], F32, tag="prod")
                 nc.vector.tensor_tensor(out=prod[:], in0=sY[:], in1=sToT[:], op=ALU.mult)
                 psc = psS.tile([P, 1], F32, tag="psc")
                 nc.tensor.matmul(out=psc[:], lhsT=prod[:], rhs=ones[:], start=True, stop=True)
                 nc.vector.tensor_copy(out=scores[:, t : t + 1], in_=psc[:])

              nc.sync.dma_start(out=scores_d[:], in_=scores[:])

    nc.finalize()
    return nc


# --------------------------------------------------------------------------
# host side
# --------------------------------------------------------------------------

def _rot_elems(params):
    """params [..., 3] (phi, theta, omega) -> v0, v1, v2, v3 f32 arrays.

    m00=(v0,-v1) m01=(-v2,-v3) m10=(v2,-v3) m11=(v0,v1)
    """
    phi, tha, omg = params[..., 0], params[..., 1], params[..., 2]
    ch, sh = np.cos(tha / 2), np.sin(tha / 2)
    a, b = (phi + omg) / 2, (phi - omg) / 2
    return (
        (ch * np.cos(a)).astype(np.float32),
        (ch * np.sin(a)).astype(np.float32),
        (sh * np.cos(b)).astype(np.float32),
        (sh * np.sin(b)).astype(np.float32),
    )


def _gate_params(par):
    """par [..., 4, 6, 3] -> [..., 24, 3] in device gate order."""
    shp = par.shape[:-3]
    return par.reshape(shp + (24, 3))


def _host_prep(entity_params, relation_params, s_idx, p_idx, o_idx):
    ent = np.asarray(entity_params, dtype=np.float32)
    rel = np.asarray(relation_params, dtype=np.float32)
    s_idx = np.asarray(s_idx)
    p_idx = np.asarray(p_idx)
    o_idx = np.asarray(o_idx)

    # ---- entity shards ----
    ent_flat = ent.reshape(E, 72)
    ent_pad = np.zeros((EPAD, 72), np.float32)
    ent_pad[:E] = ent_flat
    ent_shards = [
        ent_pad[c * EPC : (c + 1) * EPC].reshape(ETILES, P, 72) for c in range(NCORES)
    ]

    # ---- sort batch by relation, build 128-elem single-relation tiles ----
    order = np.argsort(p_idx, kind="stable")
    tiles = []  # (rel, elem_positions array of len<=128)
    bounds = np.searchsorted(p_idx[order], np.arange(R + 1))
    for r in range(R):
        grp = order[bounds[r] : bounds[r + 1]]
        for i in range(0, len(grp), P):
            tiles.append((r, grp[i : i + P]))
    ntiles = len(tiles)
    assert ntiles <= NCORES * NT, f"too many tiles {ntiles}"

    # contiguous split across cores
    per = -(-ntiles // NCORES)
    core_tiles = [tiles[c * per : (c + 1) * per] for c in range(NCORES)]

    gate_rel = _gate_params(rel)  # [R, 24, 3]
    v0, v1, v2, v3 = _rot_elems(gate_rel)  # each [R, 24]

    in_maps = []
    outpos = np.full((NCORES, NT, P), -1, np.int64)
    for c in range(NCORES):
        ct = core_tiles[c]
        rels = sorted({r for r, _ in ct})
        assert len(rels) <= RSLOT, f"core {c} has {len(rels)} relations"
        slot_of = {r: i for i, r in enumerate(rels)}

        sidx = np.zeros((NT, P), np.int32)
        oidx = np.zeros((NT, P), np.int32)
        widx = np.zeros((NT, P), np.int32)
        for t, (r, elems) in enumerate(ct):
            n = len(elems)
            sidx[t, :n] = s_idx[elems]
            oidx[t, :n] = o_idx[elems]
            widx[t, :] = slot_of[r] * P + np.arange(P)
            outpos[c, t, :n] = elems
        for t in range(len(ct), NT):
            widx[t, :] = np.arange(P)

        # W coeff table [P, WTILES, NGC]: partition half = rel slot 2wt(+1)
        wcoef = np.zeros((P, WTILES, NGC), np.float32)
        wfac = np.zeros((P, WTILES, 6, 6), np.float32)
        for sl, r in enumerate(rels):
            wt, half = divmod(sl, 2)
            rows = slice(64 * half, 64 * half + 64)
            # CRot gates g=6..23 use slots at g*NCO
            for g in range(24):
                sl7 = g * NCO
                vals = (v0[r, g], v1[r, g], v2[r, g], v3[r, g],
                        -v1[r, g], -v2[r, g], -v3[r, g])
                for k, vv in enumerate(vals):
                    wcoef[rows, wt, sl7 + k] = vv
            # layer-0 doubling factors: step k expands wire q=5-k
            # basis j (0..63): bit of wire q = (j >> (5-q)) & 1
            j = np.arange(64)
            for k in range(6):
                qg = 5 - k  # wire index == layer-0 gate index
                bit = (j >> (5 - qg)) & 1  # == (j >> k) & 1
                # u columns: u[:,0] = (m00, m10), u[:,1] = (m01, m11)
                m00 = (v0[r, qg], -v1[r, qg])
                m01 = (-v2[r, qg], -v3[r, qg])
                m10 = (v2[r, qg], -v3[r, qg])
                m11 = (v0[r, qg], v1[r, qg])
                u0 = np.where(bit == 0, m00[0], m01[0]), np.where(bit == 0, m00[1], m01[1])
                u1 = np.where(bit == 0, m10[0], m11[0]), np.where(bit == 0, m10[1], m11[1])
                wfac[rows, wt, k, 0] = u0[0]
                wfac[rows, wt, k, 1] = u0[1]
                wfac[rows, wt, k, 2] = -u0[1]
                wfac[rows, wt, k, 3] = u1[0]
                wfac[rows, wt, k, 4] = u1[1]
                wfac[rows, wt, k, 5] = -u1[1]

        in_maps.append(
            {
                "ent_par": ent_shards[c],
                "wcoef": wcoef,
                "wfac": wfac,
                "sidx": sidx,
                "oidx": oidx,
                "widx": widx,
                "ident": np.eye(P, dtype=np.float32),
            }
        )
    return in_maps, outpos


_PROGRAM = None


def kernel(entity_params, relation_params, s_idx, p_idx, o_idx):
    global _PROGRAM
    in_maps, outpos = _host_prep(entity_params, relation_params, s_idx, p_idx, o_idx)
    if _PROGRAM is None:
        _PROGRAM = build_program()
    nc = _PROGRAM
    res = run_bass_kernel_spmd(nc, in_maps, list(range(NCORES)))
    out = np.zeros(B, np.float32)
    for c in range(NCORES):
        sc = res.results[c]["scores"]  # [P, NT]
        pos = outpos[c]  # [NT, P]
        mask = pos >= 0
        out[pos[mask]] = sc.T[mask]
    return out


if __name__ == "__main__":
    # quick smoke: build only
    build_program()
    print("build OK")



# revision 11
# speedup vs baseline: 1.7778x; 1.7778x over previous
"""Trainium2 Bass kernel for the quantum-circuit KG-embedding scoring model.

score(s,p,o) = Re(<B_o h | W_p | B_s h>), B_e / W_p = 24-gate circuit blocks,
h = |+>^6.  State dim 64 complex = 128 reals [re(64) | im(64)].

Device algorithm (8 cores, SPMD), fp16 throughout the heavy paths:

  Chain phase (DVE, fp16 tile-minor layout [128 ent, 128 amp-real, T]):
    A-chain: evolve 1280 entities/core (10 tiles innermost) through the
    entity block (product-state doubling + 18 CRots).  Every DVE op has a
    stride-1 fp16 innermost dim -> 2x perf mode; coefficient tables are
    pre-replicated over the gate's low amp bits so no op exceeds 3 free
    dims (TENSOR3D) and no gate needs group splitting.
    Store T rows fp16 -> AllGather -> T_full [10240,128] fp16.
    W-chain: same machinery on 13 tiles = 26 relation slots x 64 basis
    columns, overlapping the AllGather.  Expand to W^T rows in DRAM.

  Phase C (supertiles of 512 elements, one relation each):
    dma_gather(transpose=True) pulls Ts^T / To^T [128 state, 13312 elems]
    straight from T_full (0.34ns/descriptor on gpsimd vs ~12ns for
    per-row indirect DMA; transfers spread over 16 DMA engines).
    dma_gather pulls per-supertile W^T [128,128] tiles.
    Per supertile: PE fp16 matmul Y = W @ Ts^T (psum f32), DVE
    prod = Y * To^T (fp16), PE ones-matmul column-sum -> scores.

Host does only: trig for the 200-relation tables, index sort/packing,
output unpermute (same division of labour as the reference baseline).
"""

import sys
import numpy as np

for _p in ("/opt/trn_rl_repo",):
    if _p not in sys.path:
        sys.path.insert(0, _p)

import concourse.bass as bass
import concourse.bacc as bacc
import concourse.mybir as mybir
from concourse import tile
from concourse.bass_utils import run_bass_kernel_spmd

F32 = mybir.dt.float32
F16 = mybir.dt.float16
I16 = mybir.dt.int16
ALU = mybir.AluOpType
ACTFN = mybir.ActivationFunctionType

P = 128
Q = 6
NA = 64                      # 2^Q amplitudes
NCORES = 8
E, R, B = 10000, 200, 65536
ETILES = 10                  # entity tiles per core
EPC = ETILES * P             # 1280 entities per core
EPAD = EPC * NCORES          # 10240 padded entity rows
WT = 13                      # W-chain tiles per core (2 rel slots each)
RSLOT = 2 * WT               # 26 relation slots per core
NST = 26                     # phase-C supertiles per core
STW = 512                    # supertile width (elements)
NIDX_T = NST * STW           # 13312 gathered T rows per table per core
NIDX_W = NST * P             # 3328 gathered W^T rows per core
R2 = float(2.0 ** -0.5)
PI = float(np.pi)

# CRot gate list: (control, target) wire pairs, in circuit order
CROTS = [(q, (q + off) % Q) for off in (1, 2, 3) for q in range(Q)]

# per-gate geometry: amp bit positions cpos=5-c (control), tpos=5-t
_GEO = []
for (c, t) in CROTS:
    cpos, tpos = 5 - c, 5 - t
    hi, lo = max(cpos, tpos), min(cpos, tpos)
    A = 1 << (5 - hi)
    Bm = 1 << (hi - lo - 1)
    C = 1 << lo
    _GEO.append((cpos, tpos, hi, lo, A, Bm, C))

# replicated coefficient table layout: per gate, 7 slots each replicated C
# times: block [7, C, T] at offset OFF[g] (in slot-columns of width T)
_OFF = []
_o = 0
for (_, _, _, _, _, _, C) in _GEO:
    _OFF.append(_o)
    _o += 7 * C
NREP = _o                    # total replicated slot-columns (= 7 * sum C)

# coefficient slot roles: 0:v0 1:v1 2:v2 3:v3 4:-v1 5:-v2 6:-v3
# output quarter <- sum of (slot, input quarter):
COMP_TERMS = {
    "a0r": [(0, "a0r"), (1, "a0i"), (5, "a1r"), (3, "a1i")],
    "a0i": [(4, "a0r"), (0, "a0i"), (6, "a1r"), (5, "a1i")],
    "a1r": [(2, "a0r"), (3, "a0i"), (0, "a1r"), (4, "a1i")],
    "a1i": [(6, "a0r"), (2, "a0i"), (1, "a1r"), (0, "a1i")],
}
QKEYS = {"a0r": (0, 0), "a0i": (1, 0), "a1r": (0, 1), "a1i": (1, 1)}


def _mk_ap(src_ap, dims):
    """Manual AP with explicit [stride, count] dims (partition dim first)."""
    return bass.AP(tensor=src_ap.tensor, offset=src_ap.offset,
                   ap=[list(d) for d in dims])


def _quarter_aps(st_ap, g, T):
    """Quarter-slice APs of state [128, 128, T] for CRot gate g.

    Returns dict name -> AP with dims [p, (A?), (Bm?), C*T] (c merged with
    the innermost tile dim; requires the state tile's last dim == T exactly).
    """
    cpos, tpos, hi, lo, A, Bm, C = _GEO[g]
    cbit_is_x = (cpos == hi)
    pdim = list(st_ap.ap[0])
    base_off = st_ap.offset
    # element strides within one partition (state tile is [128, 128, T],
    # contiguous): amp stride = T, tile stride = 1
    s_r = 64 * T
    s_a = 2 * Bm * 2 * C * T
    s_x = Bm * 2 * C * T
    s_b = 2 * C * T
    s_y = C * T
    out = {}
    for name, (r, tval) in QKEYS.items():
        xbit, ybit = (1, tval) if cbit_is_x else (tval, 1)
        off = base_off + r * s_r + xbit * s_x + ybit * s_y
        dims = [pdim]
        if A > 1:
            dims.append([s_a, A])
        if Bm > 1:
            dims.append([s_b, Bm])
        dims.append([1, C * T])
        out[name] = _mk_ap(st_ap, dims)
        out[name] = bass.AP(tensor=st_ap.tensor, offset=off,
                            ap=out[name].ap)
    return out


def _coef_aps(coef_ap, g, T):
    """Slot APs (broadcast to quarter shape) from replicated coef table
    [128, NREP, T].  Slot k of gate g occupies columns OFF[g]+k*C ..
    +C, real memory, so the merged (C*T) innermost dim is stride-1."""
    cpos, tpos, hi, lo, A, Bm, C = _GEO[g]
    pdim = list(coef_ap.ap[0])
    out = []
    for k in range(7):
        off = coef_ap.offset + (_OFF[g] + k * C) * T
        dims = [pdim]
        if A > 1:
            dims.append([0, A])
        if Bm > 1:
            dims.append([0, Bm])
        dims.append([1, C * T])
        out.append(bass.AP(tensor=coef_ap.tensor, offset=off, ap=dims))
    return out


def _emit_crot(nc, pool, cur, nxt, coef, g, T, tag):
    """One CRot gate: read cur, write nxt (ping-pong), fp16 tile-minor."""
    cpos, tpos, hi, lo, A, Bm, C = _GEO[g]
    qc = _quarter_aps(cur[:], g, T)
    qn = _quarter_aps(nxt[:], g, T)
    co = _coef_aps(coef[:], g, T)
    fsz = A * Bm * C * T                       # quarter free size
    for name, terms in COMP_TERMS.items():
        m1t = pool.tile([P, fsz], F16, tag=f"{tag}m1")
        m2t = pool.tile([P, fsz], F16, tag=f"{tag}m2")
        m3t = pool.tile([P, fsz], F16, tag=f"{tag}m3")
        shape_dims = qc[name].ap[1:]

        def shaped(tile_t):
            dims = [list(tile_t[:].ap[0])]
            stride = 1
            rev = []
            for d in reversed(shape_dims):
                rev.append([stride, d[1]])
                stride *= d[1]
            dims += rev[::-1]
            return bass.AP(tensor=tile_t[:].tensor, offset=tile_t[:].offset,
                           ap=dims)
        m1, m2, m3 = shaped(m1t), shaped(m2t), shaped(m3t)
        (s0, i0), (s1, i1), (s2, i2), (s3, i3) = terms
        nc.vector.tensor_tensor(out=m1, in0=qc[i0], in1=co[s0], op=ALU.mult)
        nc.vector.tensor_tensor(out=m2, in0=qc[i1], in1=co[s1], op=ALU.mult)
        nc.vector.tensor_tensor(out=m1, in0=m1, in1=m2, op=ALU.add)
        nc.vector.tensor_tensor(out=m2, in0=qc[i2], in1=co[s2], op=ALU.mult)
        nc.vector.tensor_tensor(out=m3, in0=qc[i3], in1=co[s3], op=ALU.mult)
        nc.vector.tensor_tensor(out=m2, in0=m2, in1=m3, op=ALU.add)
        nc.vector.tensor_tensor(out=qn[name], in0=m1, in1=m2, op=ALU.add)
    # inactive (control=0) half: single merged copy cur -> nxt on ACT
    u = 64 >> cpos                              # dims above cpos incl r
    m = 1 << cpos
    s_c = m * T
    for st_ap, dst in ((cur[:], 0), (nxt[:], 1)):
        dims = [list(st_ap.ap[0])]
        if u > 1:
            dims.append([2 * s_c, u])
        dims.append([1, m * T])
        ap = bass.AP(tensor=st_ap.tensor, offset=st_ap.offset, ap=dims)
        if dst == 0:
            src_ap = ap
        else:
            dst_ap = ap
    nc.scalar.activation(out=dst_ap, in_=src_ap, func=ACTFN.Copy)


def _emit_doubling(nc, pool, sbufs, fac, T, tag):
    """Product state from factors, tile-minor.

    sbufs: (sA, sB) [128, 128, T] fp16.  fac: [128, 6, 6, T] fp16 with
    per-step slots [u0r, u0i, -u0i, u1r, u1i, -u1i]; step k expands wire
    q=5-k.  Returns the buffer holding the result (sA).
    """
    sA, sB = sbufs
    cur = sA
    for (dst_col, src_slot) in ((0, 0), (1, 3)):       # re: u0r, u1r
        nc.vector.tensor_copy(out=cur[:, dst_col, :],
                              in_=fac[:, 0, src_slot, :])
    for (dst_col, src_slot) in ((64, 1), (65, 4)):     # im: u0i, u1i
        nc.vector.tensor_copy(out=cur[:, dst_col, :],
                              in_=fac[:, 0, src_slot, :])
    for k in range(1, 6):
        w = 1 << k
        nxt = sB if cur is sA else sA
        cr, ci = cur[:, 0:w, :], cur[:, 64:64 + w, :]
        for m in (0, 1):
            fr = fac[:, k, 3 * m, :].unsqueeze(1).to_broadcast([P, w, T])
            fi = fac[:, k, 3 * m + 1, :].unsqueeze(1).to_broadcast([P, w, T])
            nfi = fac[:, k, 3 * m + 2, :].unsqueeze(1).to_broadcast([P, w, T])
            dr = nxt[:, m * w:m * w + w, :]
            di = nxt[:, 64 + m * w:64 + m * w + w, :]
            t1 = pool.tile([P, w * T], F16, tag=tag + "a")
            t2 = pool.tile([P, w * T], F16, tag=tag + "b")
            t1v = t1[:].rearrange("p (w t) -> p w t", w=w)
            t2v = t2[:].rearrange("p (w t) -> p w t", w=w)
            nc.vector.tensor_tensor(out=t1v, in0=cr, in1=fr, op=ALU.mult)
            nc.vector.tensor_tensor(out=t2v, in0=ci, in1=nfi, op=ALU.mult)
            nc.vector.tensor_tensor(out=dr, in0=t1v, in1=t2v, op=ALU.add)
            nc.vector.tensor_tensor(out=t1v, in0=cr, in1=fi, op=ALU.mult)
            nc.vector.tensor_tensor(out=t2v, in0=ci, in1=fr, op=ALU.mult)
            nc.vector.tensor_tensor(out=di, in0=t1v, in1=t2v, op=ALU.add)
        cur = nxt
    if cur is not sA:
        nc.vector.tensor_copy(out=sA[:], in_=cur[:])
    return sA


def _emit_coef_replicate(nc, base, coef, T):
    """base [128, 18, 7, T] -> replicated coef [128, NREP, T]."""
    cv = coef[:]
    for g in range(18):
        C = _GEO[g][6]
        src = base[:, g, :, :].unsqueeze(2).to_broadcast([P, 7, C, T])
        dst = bass.AP(tensor=cv.tensor, offset=cv.offset + _OFF[g] * T,
                      ap=[list(cv.ap[0]), [C * T, 7], [T, C], [1, T]])
        nc.vector.tensor_copy(out=dst, in_=src)


_STAGES = ["prep", "dblA", "chainA", "storeT", "chainW", "storeW", "gather", "gatherw", "full"]


def build_program(stop_after=None, no_collective=False):
    lim = _STAGES.index(stop_after) if stop_after else len(_STAGES)

    def on(stage):
        return _STAGES.index(stage) < lim or stage == stop_after
    nc = bacc.Bacc("TRN2", target_bir_lowering=False, debug=False,
                   num_swdge_queues=4)

    ent = nc.dram_tensor("ent_par", [ETILES, P, 72], F32, kind="ExternalInput")
    wcoef_d = nc.dram_tensor("wcoefb", [P, 18, 7, WT], F16, kind="ExternalInput")
    wfac_d = nc.dram_tensor("wfac", [P, 6, 6, WT], F16, kind="ExternalInput")
    sidx_d = nc.dram_tensor("sidx", [P, NIDX_T // 16], I16, kind="ExternalInput")
    oidx_d = nc.dram_tensor("oidx", [P, NIDX_T // 16], I16, kind="ExternalInput")
    widx_d = nc.dram_tensor("widx", [P, NIDX_W // 16], I16, kind="ExternalInput")
    scores_d = nc.dram_tensor("scores", [1, NST * STW], F32, kind="ExternalOutput")
    dbg_d = nc.dram_tensor("dbg", [P, 8192], F16, kind="ExternalOutput") if stop_after else None

    with tile.TileContext(nc) as tc:
        with (
            tc.tile_pool(name="const", bufs=1) as cp,
            tc.tile_pool(name="gtmp", bufs=2) as gp,
            tc.tile_pool(name="state", bufs=1) as sp,
            tc.tile_pool(name="cbuf", bufs=1) as cb,
            tc.tile_pool(name="prodb", bufs=3) as pb,
            tc.tile_pool(name="cpy", bufs=2, space="PSUM") as psY,
            tc.tile_pool(name="cpsc", bufs=2, space="PSUM") as psS,
            tc.tile_pool(name="dram", bufs=1, space="DRAM") as dp,
        ):
            # ---------------- DRAM scratch ----------------
            T_loc = dp.tile([EPC, P], F16)
            T_full = dp.tile([EPAD, P], F16, addr_space="Shared")
            W_loc = dp.tile([P * RSLOT, P], F16)   # row = j*RSLOT + slot

            # ---------------- inputs ----------------
            ang = cp.tile([P, ETILES, 72], F32)
            nc.sync.dma_start(out=ang[:], in_=ent[:].rearrange("t p k -> p t k"))
            wcoefb = cp.tile([P, 18, 7, WT], F16)
            nc.sync.dma_start(out=wcoefb[:], in_=wcoef_d[:])
            facW = cp.tile([P, 6, 6, WT], F16)
            nc.sync.dma_start(out=facW[:], in_=wfac_d[:])
            sidx = cp.tile([P, NIDX_T // 16], I16)
            nc.sync.dma_start(out=sidx[:], in_=sidx_d[:])
            oidx = cp.tile([P, NIDX_T // 16], I16)
            nc.sync.dma_start(out=oidx[:], in_=oidx_d[:])
            widx = cp.tile([P, NIDX_W // 16], I16)
            nc.sync.dma_start(out=widx[:], in_=widx_d[:])

            ones = cp.tile([P, 1], F16)
            nc.vector.memset(ones[:], 1.0)

            # const APs for activation scale/bias floats
            cdb = cp.tile([P, 3], F32)
            nc.vector.memset(cdb[:, 0:1], 0.0)
            nc.vector.memset(cdb[:, 1:2], 0.5)
            nc.vector.memset(cdb[:, 2:3], PI / 2)
            nc.const_aps.aps[(F32, 0.0)] = cdb[:, 0:1]
            nc.const_aps.aps[(F32, 0.5)] = cdb[:, 1:2]
            nc.const_aps.aps[(F32, PI / 2)] = cdb[:, 2:3]

            # ---------------- A: entity angle prep ----------------
            TA = ETILES
            angT = cp.tile([P, 72, TA], F32)     # tile-minor angles
            nc.vector.tensor_copy(
                out=angT[:], in_=ang[:].rearrange("p t k -> p k t"))
            gv = angT[:].rearrange("p (g a) t -> p g a t", g=24, a=3)
            phi, tha, omg = gv[:, :, 0, :], gv[:, :, 1, :], gv[:, :, 2, :]
            s1 = cp.tile([P, 24, TA], F32)
            s2 = cp.tile([P, 24, TA], F32)
            nc.vector.tensor_tensor(out=s1[:], in0=phi, in1=omg, op=ALU.add)
            nc.vector.tensor_tensor(out=s2[:], in0=phi, in1=omg, op=ALU.subtract)

            half = cp.tile([P, 6, 24, TA], F32)
            trig = cp.tile([P, 6, 24, TA], F32)  # ch sh ca sa cb sb
            hv, tv = half[:], trig[:]
            for i, srcv in ((0, tha), (2, s1[:]), (4, s2[:])):
                nc.vector.tensor_scalar(
                    out=hv[:, i], in0=srcv, scalar1=0.5, scalar2=PI / 2,
                    op0=ALU.mult, op1=ALU.add)
                nc.vector.tensor_scalar_mul(hv[:, i + 1], srcv, 0.5)
            for i in range(6):
                nc.scalar.activation(out=tv[:, i], in_=hv[:, i], func=ACTFN.Sin)

            # products v0..v3 for all 24 gates, f32
            vprod = cp.tile([P, 4, 24, TA], F32)
            vv = vprod[:]
            nc.vector.tensor_tensor(out=vv[:, 0], in0=tv[:, 0], in1=tv[:, 2], op=ALU.mult)
            nc.vector.tensor_tensor(out=vv[:, 1], in0=tv[:, 0], in1=tv[:, 3], op=ALU.mult)
            nc.vector.tensor_tensor(out=vv[:, 2], in0=tv[:, 1], in1=tv[:, 4], op=ALU.mult)
            nc.vector.tensor_tensor(out=vv[:, 3], in0=tv[:, 1], in1=tv[:, 5], op=ALU.mult)

            # base CRot coef table [p, 18, 7, T] fp16 (chain gates 6..23)
            abase = cp.tile([P, 18, 7, TA], F16)
            ab = abase[:]
            for k in range(4):
                nc.vector.tensor_copy(out=ab[:, :, k, :], in_=vv[:, k, 6:24, :])
            for dst, src in ((4, 1), (5, 2), (6, 3)):
                nc.vector.tensor_scalar_mul(ab[:, :, dst, :], vv[:, src, 6:24, :], -1.0)

            # layer-0 doubling factors facA [p, 6 steps, 6 slots, T]
            # step k expands wire q=5-k -> uses layer-0 gate qg=5-k
            # f0r=(v0-v2)r2  f0i=-(v1+v3)r2  f1r=(v0+v2)r2  f1i=(v1-v3)r2
            facA = cp.tile([P, 6, 6, TA], F16)
            t6 = cp.tile([P, 6, TA], F32)
            t6v = t6[:]
            fav = facA[:]

            def fac_dst(slot):
                # k dim reversed: k = 5 - g  (g = gate 0..5)
                base = fav.offset + 5 * (6 * TA) + slot * TA
                return bass.AP(tensor=fav.tensor, offset=base,
                               ap=[list(fav.ap[0]), [-6 * TA, 6], [1, TA]])
            g03 = vv[:, 0, 0:6, :], vv[:, 1, 0:6, :], vv[:, 2, 0:6, :], vv[:, 3, 0:6, :]
            v0g, v1g, v2g, v3g = g03
            nc.vector.tensor_tensor(out=t6v, in0=v0g, in1=v2g, op=ALU.subtract)
            nc.vector.tensor_scalar_mul(fac_dst(0), t6v, R2)
            nc.vector.tensor_tensor(out=t6v, in0=v1g, in1=v3g, op=ALU.add)
            nc.vector.tensor_scalar_mul(fac_dst(1), t6v, -R2)
            nc.vector.tensor_scalar_mul(fac_dst(2), t6v, R2)
            nc.vector.tensor_tensor(out=t6v, in0=v0g, in1=v2g, op=ALU.add)
            nc.vector.tensor_scalar_mul(fac_dst(3), t6v, R2)
            nc.vector.tensor_tensor(out=t6v, in0=v1g, in1=v3g, op=ALU.subtract)
            nc.vector.tensor_scalar_mul(fac_dst(4), t6v, R2)
            nc.vector.tensor_scalar_mul(fac_dst(5), t6v, -R2)

            # replicated coef tables
            acoef = cp.tile([P, NREP, TA], F16)
            _emit_coef_replicate(nc, abase, acoef, TA)
            wcoef = cp.tile([P, NREP, WT], F16)
            _emit_coef_replicate(nc, wcoefb, wcoef, WT)

            if stop_after == "prep":
                nc.sync.dma_start(out=dbg_d[:, 0:NREP * TA],
                                  in_=acoef[:].rearrange("p a t -> p (a t)"))
                nc.sync.dma_start(out=dbg_d[:, NREP * TA:NREP * TA + 360],
                                  in_=facA[:].rearrange("p a b t -> p (a b t)"))
            # ---------------- A chain ----------------
            if on("dblA"):
                sA0 = sp.tile([P, P, TA], F16)
                sA1 = sp.tile([P, P, TA], F16)
                cur = _emit_doubling(nc, gp, (sA0, sA1), facA[:], TA, "adbl")
                nxt = sA1
            if stop_after == "dblA":
                nc.sync.dma_start(out=dbg_d[:, 0:P * TA],
                                  in_=cur[:].rearrange("p a t -> p (a t)"))
            if on("chainA"):
                for g in range(18):
                    _emit_crot(nc, gp, cur, nxt, acoef, g, TA, "acr")
                    cur, nxt = nxt, cur
            if stop_after == "chainA":
                nc.sync.dma_start(out=dbg_d[:, 0:P * TA],
                                  in_=cur[:].rearrange("p a t -> p (a t)"))
            if on("storeT"):
                # T store (transpose to row-major) + AllGather
                Tout = cp.tile([P, TA, P], F16)
                nc.scalar.activation(
                    out=Tout[:], in_=cur[:].rearrange("p a t -> p t a"),
                    func=ACTFN.Copy)
                nc.sync.dma_start(
                    out=T_loc[:].rearrange("(t p) k -> p t k", p=P), in_=Tout[:])
                if no_collective:
                    nc.sync.dma_start(out=T_full[0:EPC, :], in_=T_loc[:])
                else:
                    nc.gpsimd.collective_compute(
                        "AllGather", ALU.bypass,
                        ins=[T_loc[:]], outs=[T_full[:]],
                        replica_groups=[list(range(NCORES))],
                    )
                    # gpsimd-engine fence on CC completion: the dma_gather
                    # chunks below otherwise race the AllGather's writes.
                    ccgate = cp.tile([P, 1], F16)
                    nc.gpsimd.dma_start(out=ccgate[:], in_=T_full[0:P, 0:1])
            if stop_after == "storeT":
                nc.sync.dma_start(out=dbg_d[:, 0:TA * P],
                                  in_=Tout[:].rearrange("p a t -> p (a t)"))

            # ---------------- W chain ----------------
            if on("chainW"):
                sW0 = sp.tile([P, P, WT], F16)
                sW1 = sp.tile([P, P, WT], F16)
                curw = _emit_doubling(nc, gp, (sW0, sW1), facW[:], WT, "wdbl")
                nxtw = sW1
                for g in range(18):
                    _emit_crot(nc, gp, curw, nxtw, wcoef, g, WT, "wcr")
                    curw, nxtw = nxtw, curw
            if stop_after == "chainW":
                nc.sync.dma_start(out=dbg_d[:, 0:P * WT],
                                  in_=curw[:].rearrange("p a t -> p (a t)"))

            # expand to W^T rows.  state partition p=(h,j): column j of W for
            # slot 2t+h, values [yr(64)|yi(64)].  W^T row j = state row;
            # W^T row 64+j = [-yi | yr].
            if on("storeW"):
                Wt = cp.tile([P, WT, P], F16)
                nc.scalar.activation(
                    out=Wt[:], in_=curw[:].rearrange("p a t -> p t a"),
                    func=ACTFN.Copy)
                Bt = cp.tile([P, WT, P], F16)
                nc.vector.tensor_scalar_mul(Bt[:, :, 0:64], Wt[:, :, 64:128], -1.0)
                nc.vector.tensor_copy(out=Bt[:, :, 64:128], in_=Wt[:, :, 0:64])
                # W_loc row = j*RSLOT + (2t+h);  p = h*64+j
                wl = W_loc[:].rearrange("(j t2 h) i -> h j t2 i", j=P, t2=WT, h=2)
                for h in (0, 1):
                    nc.sync.dma_start(out=wl[h, 0:64], in_=Wt[64 * h:64 * h + 64])
                    nc.sync.dma_start(out=wl[h, 64:128], in_=Bt[64 * h:64 * h + 64])

            # ---------------- phase C gathers (chunked; SWDGE ring caps
            # one gather at ~896 idxs) ----------------
            GCH = 896
            qn = [0]

            def gather_chunks(out3, idxs_t, src, nidx, transpose):
                s = 0
                while s < nidx:
                    n = min(GCH, nidx - s)
                    if transpose:
                        o = out3[:, :, s:s + n]
                    else:
                        o = out3[:, s // P:(s + n) // P, :]
                    nc.gpsimd.dma_gather(
                        out_ap=o, in_ap=src, idxs_ap=idxs_t[:, s // 16:(s + n) // 16],
                        num_idxs=n, num_idxs_reg=n, elem_size=P,
                        transpose=transpose, queue_num=qn[0])
                    qn[0] = 0
                    s += n

            if on("gather"):
              TsT = cb.tile([P, 1, NIDX_T], F16, tag="tst")
              gather_chunks(TsT[:], sidx, T_full[:], NIDX_T, True)
              ToT = cb.tile([P, 1, NIDX_T], F16, tag="tot")
              gather_chunks(ToT[:], oidx, T_full[:], NIDX_T, True)
              Wg = cb.tile([P, NST, P], F16, tag="wg")
              gather_chunks(Wg[:], widx, W_loc[:], NIDX_W, False)
            if stop_after == "gather":
                nc.sync.dma_start(out=dbg_d[:, 0:2048],
                                  in_=TsT[:, 0, 0:2048])
            if stop_after == "gatherw":
                nc.sync.dma_start(out=dbg_d[:, 0:3328],
                                  in_=Wg[:].rearrange("p a b -> p (a b)"))

            if on("full"):
              scores = cp.tile([1, NST * STW], F32)
              for st in range(NST):
                 pY = psY.tile([P, STW], F32, tag="py")
                 nc.tensor.matmul(out=pY[:], lhsT=Wg[:, st, :],
                                  rhs=TsT[:, 0, st * STW:(st + 1) * STW],
                                  start=True, stop=True)
                 prod = pb.tile([P, STW], F16, tag="prod")
                 nc.vector.tensor_tensor(
                     out=prod[:], in0=pY[:],
                     in1=ToT[:, 0, st * STW:(st + 1) * STW], op=ALU.mult)
                 pS = psS.tile([1, STW], F32, tag="ps")
                 nc.tensor.matmul(out=pS[:], lhsT=ones[:], rhs=prod[:],
                                  start=True, stop=True)
                 nc.scalar.activation(
                     out=scores[0:1, st * STW:(st + 1) * STW], in_=pS[:],
                     func=ACTFN.Copy)
              nc.sync.dma_start(out=scores_d[:], in_=scores[:])

    nc.finalize()
    return nc


# --------------------------------------------------------------------------
# host side
# --------------------------------------------------------------------------

def _rot_elems(params):
    """params [..., 3] (phi, theta, omega) -> v0..v3 f32.

    m00=(v0,-v1) m01=(-v2,-v3) m10=(v2,-v3) m11=(v0,v1)
    """
    phi, tha, omg = params[..., 0], params[..., 1], params[..., 2]
    ch, sh = np.cos(tha / 2), np.sin(tha / 2)
    a, b = (phi + omg) / 2, (phi - omg) / 2
    return (
        (ch * np.cos(a)).astype(np.float32),
        (ch * np.sin(a)).astype(np.float32),
        (sh * np.cos(b)).astype(np.float32),
        (sh * np.sin(b)).astype(np.float32),
    )


def _pack_idxs(flat):
    """int array [n] (n % 16 == 0) -> [128, n/16] int16 (replicated x8)."""
    n = len(flat)
    blk = np.asarray(flat, np.int16).reshape(n // 16, 16).T
    return np.tile(blk, (8, 1))


def _host_prep(entity_params, relation_params, s_idx, p_idx, o_idx):
    ent = np.asarray(entity_params, dtype=np.float32)
    rel = np.asarray(relation_params, dtype=np.float32)
    s_idx = np.asarray(s_idx)
    p_idx = np.asarray(p_idx)
    o_idx = np.asarray(o_idx)

    # ---- entity shards ----
    ent_flat = ent.reshape(E, 72)
    ent_pad = np.zeros((EPAD, 72), np.float32)
    ent_pad[:E] = ent_flat
    ent_shards = [
        ent_pad[c * EPC:(c + 1) * EPC].reshape(ETILES, P, 72)
        for c in range(NCORES)
    ]

    # ---- supertiles (relation, <=512 elements) ----
    order = np.argsort(p_idx, kind="stable")
    bounds = np.searchsorted(p_idx[order], np.arange(R + 1))
    tiles = []
    for r in range(R):
        grp = order[bounds[r]:bounds[r + 1]]
        for i in range(0, len(grp), STW):
            tiles.append((r, grp[i:i + STW]))
    ntiles = len(tiles)
    assert ntiles <= NCORES * NST, f"too many supertiles {ntiles}"

    # greedy balance: relations (grouped) to cores, bounded slots+tiles
    by_rel = {}
    for t in tiles:
        by_rel.setdefault(t[0], []).append(t)
    core_tiles = [[] for _ in range(NCORES)]
    core_rels = [set() for _ in range(NCORES)]
    for r, ts in sorted(by_rel.items(), key=lambda kv: -len(kv[1])):
        remaining = list(ts)
        while remaining:
            cands = sorted(
                range(NCORES),
                key=lambda c: (len(core_tiles[c]), len(core_rels[c])))
            placed = False
            for c in cands:
                cap = NST - len(core_tiles[c])
                if cap <= 0 or len(core_rels[c]) >= RSLOT:
                    continue
                take = remaining[:cap]
                core_tiles[c].extend(take)
                core_rels[c].add(r)
                remaining = remaining[len(take):]
                placed = True
                break
            assert placed, "balance failure"

    gate_rel = rel.reshape(R, 24, 3)
    v0, v1, v2, v3 = _rot_elems(gate_rel)   # each [R, 24]

    in_maps = []
    outpos = np.full((NCORES, NST, STW), -1, np.int64)
    for c in range(NCORES):
        ct = core_tiles[c]
        rels = sorted(core_rels[c])
        slot_of = {r: i for i, r in enumerate(rels)}
        assert len(rels) <= RSLOT

        sflat = np.zeros(NIDX_T, np.int64)
        oflat = np.zeros(NIDX_T, np.int64)
        wflat = np.zeros(NIDX_W, np.int64)
        for t in range(NST):
            if t < len(ct):
                r, elems = ct[t]
                n = len(elems)
                sflat[t * STW:t * STW + n] = s_idx[elems]
                oflat[t * STW:t * STW + n] = o_idx[elems]
                outpos[c, t, :n] = elems
                slot = slot_of[r]
            else:
                slot = 0
            # W_loc row = j * RSLOT + slot
            wflat[t * P:(t + 1) * P] = np.arange(P) * RSLOT + slot

        # W-chain coef [p, 18, 7, WT] fp16 + doubling factors [p,6,6,WT]
        wcoefb = np.zeros((P, 18, 7, WT), np.float16)
        wfac = np.zeros((P, 6, 6, WT), np.float16)
        for sl, r in enumerate(rels):
            wt, hh = divmod(sl, 2)
            rows = slice(64 * hh, 64 * hh + 64)
            for g in range(18):
                gg = 6 + g
                vals = (v0[r, gg], v1[r, gg], v2[r, gg], v3[r, gg],
                        -v1[r, gg], -v2[r, gg], -v3[r, gg])
                for k, vvv in enumerate(vals):
                    wcoefb[rows, g, k, wt] = vvv
            j = np.arange(64)
            for k in range(6):
                qg = 5 - k
                bit = (j >> k) & 1
                m00 = (v0[r, qg], -v1[r, qg])
                m01 = (-v2[r, qg], -v3[r, qg])
                m10 = (v2[r, qg], -v3[r, qg])
                m11 = (v0[r, qg], v1[r, qg])
                u0r = np.where(bit == 0, m00[0], m01[0])
                u0i = np.where(bit == 0, m00[1], m01[1])
                u1r = np.where(bit == 0, m10[0], m11[0])
                u1i = np.where(bit == 0, m10[1], m11[1])
                wfac[rows, k, 0, wt] = u0r
                wfac[rows, k, 1, wt] = u0i
                wfac[rows, k, 2, wt] = -u0i
                wfac[rows, k, 3, wt] = u1r
                wfac[rows, k, 4, wt] = u1i
                wfac[rows, k, 5, wt] = -u1i

        in_maps.append({
            "ent_par": ent_shards[c],
            "wcoefb": wcoefb,
            "wfac": wfac,
            "sidx": _pack_idxs(sflat),
            "oidx": _pack_idxs(oflat),
            "widx": _pack_idxs(wflat),
        })
    return in_maps, outpos


_PROGRAM = None


def kernel(entity_params, relation_params, s_idx, p_idx, o_idx):
    global _PROGRAM
    in_maps, outpos = _host_prep(entity_params, relation_params,
                                 s_idx, p_idx, o_idx)
    if _PROGRAM is None:
        _PROGRAM = build_program()
    nc = _PROGRAM
    res = run_bass_kernel_spmd(nc, in_maps, list(range(NCORES)))
    out = np.zeros(B, np.float32)
    for c in range(NCORES):
        sc = res.results[c]["scores"].reshape(NST, STW)
        pos = outpos[c]
        mask = pos >= 0
        out[pos[mask]] = sc[mask]
    return out


if __name__ == "__main__":
    build_program()
    print("build OK")
